# revision 5
# baseline (speedup 1.0000x reference)
"""Trainium2 Bass kernel for nn_KernelToeplitzCausalLinear.

Computes, for x (B=8, E=2048, S=1024), weight (4, 1024), bias (1024,):

    out[b, e, t] = sum_k sum_{s<=t} x[b, e+k-3, s] * weight[k, t-s] + bias[t]

i.e. a causal 4-tap shift along E combined with a full causal (upper-
triangular Toeplitz) matmul along the dim axis.

Sharding: data-parallel over batch B -> one NeuronCore per batch element
(shifts stay within a batch element; the small weight is replicated).

Per-core kernel (bf16 matmuls, fp32 PSUM accumulation):
  The 128x128-blocked Toeplitz operator has only 32 distinct blocks
  T[k,d] (d = tblock - sblock >= 0).  These are made the STATIONARY
  operand; the MOVING operand is a 512-wide slice of the transposed
  input x^T, so the kernel computes out^T (t on partitions, e on the
  free axis) and the host transposes back.  This eliminates all on-chip
  transposes (the host supplies x^T with 3 zero columns of left-padding
  so the four E-taps become free-axis offsets k) and maximizes the
  moving-stream width per instruction (PSUM bank limit: 512 fp32).

  For each t-block tb (8) and 512-wide e-chunk (4): one PSUM bank
  accumulates 4*(tb+1) matmuls (only sb <= tb contributes - block-
  triangular structure, 44% of the dense FLOPs).  The moving operand is
  fp8-e3m4 (x prescaled by 2 on host; verified device rel err 1.14e-2
  vs the 2e-2 gate) while the stationary W blocks stay bf16 - the 8-bit
  moving stream halves input DMA and reduces PE toggle power, which this
  part throttles on.  DVE evicts PSUM with out = ps/2 + bias into bf16;
  DMA out.  576 matmuls total.

  Weights are DMA'd grouped by diagonal d so tb=0 can start after
  ~2us; x^T strips are double-buffered so back-to-back invocations
  pipeline without a write-after-read stall on the input DMA.
"""
import numpy as np
from contextlib import ExitStack

import ml_dtypes

import concourse.bass as bass
import concourse.tile as tile
from concourse import bacc, mybir
from concourse.bass_utils import run_bass_kernel_spmd

P = 128
B = 8
E = 2048
S = 1024
K = 4
NB = S // P       # 8 t/s blocks
ECW = 512
NEC = E // ECW    # 4 e-chunks
F32 = mybir.dt.float32
BF16 = mybir.dt.bfloat16
F8E3 = mybir.dt.float8e3
XSCALE = 2.0


def make_wsd(weight: np.ndarray) -> np.ndarray:
    """(4, 1024) -> (8, 128, 512) bf16, wsd[d][i, k*128+j] = T[k,d][i,j]
    = weight[k, 128*d + j - i] where valid and causal, else 0."""
    i = np.arange(P)[:, None]
    j = np.arange(P)[None, :]
    out = np.zeros((NB, P, K * P), dtype=ml_dtypes.bfloat16)
    for d in range(NB):
        idx = 128 * d + j - i
        valid = (idx >= 0) & (idx < S)
        for k in range(K):
            blk = np.where(valid, weight[k][idx.clip(0, S - 1)], 0.0)
            out[d][:, k * P:(k + 1) * P] = blk.astype(ml_dtypes.bfloat16)
    return np.ascontiguousarray(out)


def make_xt(xb: np.ndarray) -> np.ndarray:
    """(E, S) -> (S, E+3) fp8-e3m4 transpose (prescaled by XSCALE) with 3
    zero left-pad columns."""
    xt = np.zeros((S, E + 3), dtype=ml_dtypes.float8_e3m4)
    xt[:, 3:] = (xb.T * XSCALE).astype(ml_dtypes.float8_e3m4)
    return xt


def make_biasc(bias: np.ndarray) -> np.ndarray:
    """(1024,) -> (128, 8) f32, biasc[p, tb] = bias[tb*128 + p]."""
    return np.ascontiguousarray(bias.reshape(NB, P).T.astype(np.float32))


def build_nc(reps: int = 1):
    nc = bacc.Bacc("TRN2", target_bir_lowering=False, debug=False)
    xt_d = nc.dram_tensor("xt", [S, E + 3], F8E3, kind="ExternalInput").ap()
    w_d = nc.dram_tensor("wsd", [NB, P, K * P], BF16, kind="ExternalInput").ap()
    b_d = nc.dram_tensor("biasc", [P, NB], F32, kind="ExternalInput").ap()
    o_d = nc.dram_tensor("outT", [S, E], BF16, kind="ExternalOutput").ap()

    with tile.TileContext(nc) as tc, ExitStack() as ctx:
        consts = ctx.enter_context(tc.tile_pool(name="consts", bufs=1))
        xt_pool = ctx.enter_context(tc.tile_pool(name="xtp", bufs=2))
        osb_pool = ctx.enter_context(tc.tile_pool(name="osb", bufs=4))
        psum = ctx.enter_context(tc.tile_pool(name="psum", bufs=8, space="PSUM"))

        biasc = consts.tile([P, NB], F32)
        nc.sync.dma_start(biasc[:], b_d[:])
        WSD = []
        for d in range(NB):
            t = consts.tile([P, K * P], BF16, name=f"wsd{d}")
            nc.sync.dma_start(t[:], w_d[d])
            WSD.append(t)

        def body(_iv=None):
            XT = [xt_pool.tile([P, E + 3], F8E3, name=f"xt{sb}")
                  for sb in range(NB)]
            for sb in range(NB):
                nc.sync.dma_start(XT[sb][:], xt_d[sb * P:(sb + 1) * P, :])
            for tb in range(NB):
                mms = [(k, sb) for sb in range(tb, -1, -1) for k in range(K)]
                for ec in range(NEC):
                    c0 = ec * ECW
                    ps = psum.tile([P, ECW], F32, name="ps")
                    for i, (k, sb) in enumerate(mms):
                        d = tb - sb
                        nc.tensor.matmul(
                            ps[:],
                            WSD[d][:, k * P:(k + 1) * P],
                            XT[sb][:, k + c0: k + c0 + ECW],
                            start=(i == 0),
                            stop=(i == len(mms) - 1),
                        )
                    ob = osb_pool.tile([P, ECW], BF16, name="ob")
                    nc.vector.tensor_scalar(
                        ob[:], ps[:], 1.0 / XSCALE, biasc[:, tb:tb + 1],
                        mybir.AluOpType.mult, mybir.AluOpType.add)
                    nc.sync.dma_start(
                        o_d[tb * P:(tb + 1) * P, c0:c0 + ECW], ob[:])

        if reps == 1:
            body()
        else:
            with tc.For_i(0, reps, 1):
                body()

    nc.compile()
    return nc


_NC_CACHE = {}


def _get_nc():
    if 'nc' not in _NC_CACHE:
        _NC_CACHE['nc'] = build_nc(1)
    return _NC_CACHE['nc']


def host_prep(x, weight, bias):
    wsd = make_wsd(np.asarray(weight, np.float32))
    biasc = make_biasc(np.asarray(bias, np.float32))
    return [
        {"xt": make_xt(np.asarray(x[b], np.float32)), "wsd": wsd,
         "biasc": biasc}
        for b in range(x.shape[0])
    ]


def kernel(x: np.ndarray, weight: np.ndarray, bias: np.ndarray) -> np.ndarray:
    x = np.asarray(x, dtype=np.float32)
    weight = np.asarray(weight, dtype=np.float32)
    bias = np.asarray(bias, dtype=np.float32)
    assert x.shape == (B, E, S), x.shape
    assert weight.shape == (K, S), weight.shape
    assert bias.shape == (S,), bias.shape

    in_maps = host_prep(x, weight, bias)
    nc = _get_nc()
    res = run_bass_kernel_spmd(nc, in_maps, list(range(B)))
    out = np.stack([
        res.results[b]["outT"].astype(np.float32).T for b in range(B)
    ])
    return np.ascontiguousarray(out)
